# revision 5
# baseline (speedup 1.0000x reference)
"""Trainium2 Bass kernel for the CILRS control module (moe_routing).

Strategy
--------
Host side (numpy, part of sharding):
  * The `join` Linear feeds ONLY the 4 control heads and has no
    nonlinearity, so ``join_W @ ctrl_W1[e]`` is folded into a single
    [640, 256] weight per expert (and biases folded accordingly).
  * Rows are routed by ``command``: expert e's rows go to cores 2e and
    2e+1 (2 cores per expert, capacity 4608 rows/core, padded).  Each
    core receives ONLY its expert's weights, so the device program is
    expert-agnostic (pure SPMD).
  * p_i rows are gathered + transposed on the host so the device gets
    feature-major [512, 4608] activations (matmul contraction dim must
    sit on SBUF partitions).

Device side (per core, 4608 rows in 9 chunks of 512):
  feature-major MLP pipeline, fp32r matmuls (full PE rate at N>=512),
  bias+ReLU epilogues fused into single ACT/DVE instructions reading
  PSUM directly.
"""

import numpy as np

B = 32768
NCORES = 8
NEXPERT = 4
RPC = 4608            # rows per core (padded capacity)
CHUNK = 512
NCHUNK = RPC // CHUNK  # 9

# ---------------------------------------------------------------------------
# Weight-pack layout: one [128, WC] fp32 array per core.
# ---------------------------------------------------------------------------
VPW1 = 0                    # vp_W1 [512,128]  -> 4 k-tiles of [128,128]
VPW2 = VPW1 + 4 * 128       # vp_W2 [128,128]
VPW3 = VPW2 + 128           # vp_W3 [128,1]
SPW1 = VPW3 + 1             # sp_W1 [1,128]   (row 0 only)
SPW2 = SPW1 + 128           # sp_W2 [128,128]
SPW3 = SPW2 + 128           # sp_W3 [128,128]
W1F = SPW3 + 128            # fused head W1' [640,256] -> 5 k-tiles of [128,256]
W2F = W1F + 5 * 256         # head W2 [256,256] -> 2 k-tiles of [128,256]
W3F = W2F + 2 * 256         # head W3 [256,3]  -> 2 k-tiles of [128,3]
SPB1 = W3F + 2 * 3
SPB2 = SPB1 + 1
SPB3 = SPB2 + 1
VPB1 = SPB3 + 1
VPB2 = VPB1 + 1
VPB3 = VPB2 + 1             # row 0 only
B1F0 = VPB3 + 1             # fused head b1'[0:128]
B1F1 = B1F0 + 1             # fused head b1'[128:256]
B2F0 = B1F1 + 1
B2F1 = B2F0 + 1
B3F = B2F1 + 1              # rows 0..2
WC = B3F + 1

_CACHE = {}


# ---------------------------------------------------------------------------
# Device program
# ---------------------------------------------------------------------------
def _build_bass():
    import concourse.bacc as bacc
    import concourse.mybir as mybir
    import concourse.tile as tile

    f32r = mybir.dt.float32r
    f32 = mybir.dt.float32
    RELU = mybir.ActivationFunctionType.Relu
    ADD = mybir.AluOpType.add
    MAX = mybir.AluOpType.max

    nc = bacc.Bacc(
        "TRN2",
        target_bir_lowering=False,
        debug=False,
        enable_asserts=False,
        num_devices=NCORES,
    )
    xT_d = nc.dram_tensor("xT", [512, RPC], f32r, kind="ExternalInput")
    spd_d = nc.dram_tensor("spd", [1, RPC], f32r, kind="ExternalInput")
    wp_d = nc.dram_tensor("wp", [128, WC], f32r, kind="ExternalInput")
    vp_d = nc.dram_tensor("vp_out", [1, RPC], f32r, kind="ExternalOutput")
    act_d = nc.dram_tensor("act_out", [3, RPC], f32r, kind="ExternalOutput")

    with tile.TileContext(nc) as tc:
        with (
            tc.tile_pool(name="const", bufs=1) as constp,
            tc.tile_pool(name="io", bufs=3) as iop,
            tc.tile_pool(name="acts", bufs=2) as actp,
            tc.tile_pool(name="outs", bufs=1) as outp,
            tc.tile_pool(name="ps", bufs=6, space="PSUM") as psp,
        ):
            wp = constp.tile([128, WC], f32r)
            nc.sync.dma_start(wp[:], wp_d.ap())
            vp_all = outp.tile([1, RPC], f32r)
            act_all = outp.tile([3, RPC], f32r)

            xTr = xT_d.ap().rearrange("(k p) r -> p k r", p=128)

            def bcol(col, p0=0, pn=128):
                return wp[p0:pn, col : col + 1].bitcast(f32)

            def relu_act(out, psum, bias_col):
                nc.scalar.activation(out, psum, RELU, bias=bcol(bias_col))

            def relu_dve(out, psum, bias_col):
                nc.vector.tensor_scalar(out, psum, bcol(bias_col), 0.0, ADD, MAX)

            for c in range(NCHUNK):
                sl = slice(c * CHUNK, (c + 1) * CHUNK)
                xt = iop.tile([128, 4, CHUNK], f32r)
                nc.sync.dma_start(xt[:], xTr[:, :, sl])
                spt = iop.tile([1, CHUNK], f32r)
                nc.sync.dma_start(spt[:], spd_d.ap()[:, sl])

                # ---- speed embedding MLP: 1 -> 128 -> 128 -> 128
                ps1 = psp.tile([128, CHUNK], f32, tag="ps")
                nc.tensor.matmul(
                    ps1[:], wp[0:1, SPW1 : SPW1 + 128], spt[:], start=True, stop=True
                )
                h1s = actp.tile([128, CHUNK], f32r)
                relu_act(h1s[:], ps1[:], SPB1)
                ps2 = psp.tile([128, CHUNK], f32, tag="ps")
                nc.tensor.matmul(
                    ps2[:], wp[:, SPW2 : SPW2 + 128], h1s[:], start=True, stop=True
                )
                h2s = actp.tile([128, CHUNK], f32r)
                relu_dve(h2s[:], ps2[:], SPB2)
                ps3 = psp.tile([128, CHUNK], f32, tag="ps")
                nc.tensor.matmul(
                    ps3[:], wp[:, SPW3 : SPW3 + 128], h2s[:], start=True, stop=True
                )
                v = actp.tile([128, CHUNK], f32r)
                nc.vector.tensor_scalar_add(v[:], ps3[:], bcol(SPB3))

                # ---- speed prediction MLP: 512 -> 128 -> 128 -> 1
                psv1 = psp.tile([128, CHUNK], f32, tag="ps")
                for k in range(4):
                    nc.tensor.matmul(
                        psv1[:],
                        wp[:, VPW1 + 128 * k : VPW1 + 128 * (k + 1)],
                        xt[:, k, :],
                        start=(k == 0),
                        stop=(k == 3),
                    )
                hv1 = actp.tile([128, CHUNK], f32r)
                relu_act(hv1[:], psv1[:], VPB1)
                psv2 = psp.tile([128, CHUNK], f32, tag="ps")
                nc.tensor.matmul(
                    psv2[:], wp[:, VPW2 : VPW2 + 128], hv1[:], start=True, stop=True
                )
                hv2 = actp.tile([128, CHUNK], f32r)
                relu_dve(hv2[:], psv2[:], VPB2)
                psv3 = psp.tile([1, CHUNK], f32, tag="ps")
                nc.tensor.matmul(
                    psv3[:], wp[:, VPW3 : VPW3 + 1], hv2[:], start=True, stop=True
                )
                nc.vector.tensor_scalar_add(vp_all[0:1, sl], psv3[:], bcol(VPB3, 0, 1))

                # ---- fused control head: 640 -> 256 -> 256 -> 3
                h1p = []
                for m in range(2):
                    psh1 = psp.tile([128, CHUNK], f32, tag="ps", name=f"psh1_{m}")
                    for k in range(4):
                        nc.tensor.matmul(
                            psh1[:],
                            wp[:, W1F + 256 * k + 128 * m : W1F + 256 * k + 128 * (m + 1)],
                            xt[:, k, :],
                            start=(k == 0),
                            stop=False,
                        )
                    nc.tensor.matmul(
                        psh1[:],
                        wp[:, W1F + 256 * 4 + 128 * m : W1F + 256 * 4 + 128 * (m + 1)],
                        v[:],
                        start=False,
                        stop=True,
                    )
                    ht = actp.tile([128, CHUNK], f32r, name=f"h1p{m}")
                    if m == 0:
                        relu_act(ht[:], psh1[:], B1F0)
                    else:
                        relu_dve(ht[:], psh1[:], B1F1)
                    h1p.append(ht)
                h2p = []
                for m in range(2):
                    psh2 = psp.tile([128, CHUNK], f32, tag="ps", name=f"psh2_{m}")
                    for k in range(2):
                        nc.tensor.matmul(
                            psh2[:],
                            wp[:, W2F + 256 * k + 128 * m : W2F + 256 * k + 128 * (m + 1)],
                            h1p[k][:],
                            start=(k == 0),
                            stop=(k == 1),
                        )
                    ht2 = actp.tile([128, CHUNK], f32r, name=f"h2p{m}")
                    if m == 0:
                        relu_act(ht2[:], psh2[:], B2F0)
                    else:
                        relu_dve(ht2[:], psh2[:], B2F1)
                    h2p.append(ht2)
                psh3 = psp.tile([3, CHUNK], f32, tag="ps")
                for k in range(2):
                    nc.tensor.matmul(
                        psh3[:],
                        wp[:, W3F + 3 * k : W3F + 3 * (k + 1)],
                        h2p[k][:],
                        start=(k == 0),
                        stop=(k == 1),
                    )
                nc.vector.tensor_scalar_add(act_all[0:3, sl], psh3[:], bcol(B3F, 0, 3))

            nc.sync.dma_start(vp_d.ap(), vp_all[:])
            nc.sync.dma_start(act_d.ap(), act_all[:])

    nc.compile()
    return nc


def _get_nc():
    if "nc" not in _CACHE:
        _CACHE["nc"] = _build_bass()
    return _CACHE["nc"]


# ---------------------------------------------------------------------------
# PJRT runner (axon): jit once, execute cheaply on repeat calls.
# ---------------------------------------------------------------------------
def _get_runner():
    if "runner" in _CACHE:
        return _CACHE["runner"]

    import jax
    import concourse.mybir as mybir
    from concourse import bass2jax
    from jax.experimental.shard_map import shard_map
    from jax.sharding import Mesh, PartitionSpec

    nc = _get_nc()
    bass2jax.install_neuronx_cc_hook()

    partition_name = nc.partition_id_tensor.name if nc.partition_id_tensor else None
    in_names = []
    out_names = []
    out_avals = []
    for alloc in nc.m.functions[0].allocations:
        if not isinstance(alloc, mybir.MemoryLocationSet):
            continue
        name = alloc.memorylocations[0].name
        if alloc.kind == "ExternalInput":
            if name != partition_name:
                in_names.append(name)
        elif alloc.kind == "ExternalOutput":
            shape = tuple(alloc.tensor_shape)
            dtype = mybir.dt.np(alloc.dtype)
            out_names.append(name)
            out_avals.append(jax.core.ShapedArray(shape, dtype))
    n_params = len(in_names)
    n_outs = len(out_avals)
    all_in_names = in_names + out_names
    if partition_name is not None:
        all_in_names = all_in_names + [partition_name]

    def _body(*args):
        operands = list(args)
        if partition_name is not None:
            operands.append(bass2jax.partition_id_tensor())
        outs = bass2jax._bass_exec_p.bind(
            *operands,
            out_avals=tuple(out_avals),
            in_names=tuple(all_in_names),
            out_names=tuple(out_names),
            lowering_input_output_aliases=(),
            sim_require_finite=True,
            sim_require_nnan=True,
            nc=nc,
        )
        return tuple(outs)

    devices = jax.devices()[:NCORES]
    mesh = Mesh(np.asarray(devices), ("core",))
    donate = tuple(range(n_params, n_params + n_outs))
    sharded = jax.jit(
        shard_map(
            _body,
            mesh=mesh,
            in_specs=(PartitionSpec("core"),) * (n_params + n_outs),
            out_specs=(PartitionSpec("core"),) * n_outs,
            check_rep=False,
        ),
        donate_argnums=donate,
        keep_unused=True,
    )

    _CACHE["sharded"] = sharded
    _CACHE["in_names"] = in_names
    _CACHE["out_avals"] = out_avals

    def run(in_maps):
        concat_in = [
            np.concatenate([in_maps[c][name] for c in range(NCORES)], axis=0)
            for name in in_names
        ]
        concat_zeros = [
            np.zeros((NCORES * a.shape[0], *a.shape[1:]), a.dtype) for a in out_avals
        ]
        out_arrs = sharded(*concat_in, *concat_zeros)
        return [
            {
                name: np.asarray(out_arrs[i]).reshape(NCORES, *out_avals[i].shape)[c]
                for i, name in enumerate(out_names)
            }
            for c in range(NCORES)
        ]

    _CACHE["runner"] = run
    return run


# ---------------------------------------------------------------------------
# Host-side prep: weight folding, routing, packing
# ---------------------------------------------------------------------------
def _prepare(p_i, speed, command, weights):
    f64 = np.float64
    join_W = weights["join_W"].astype(f64)
    join_b = weights["join_b"].astype(f64)
    w1f = []
    b1f = []
    for e in range(NEXPERT):
        c1 = weights["ctrl_W1"][e].astype(f64)
        w1f.append((join_W @ c1).astype(np.float32))
        b1f.append((join_b @ c1 + weights["ctrl_b1"][e].astype(f64)).astype(np.float32))

    # shared part of the weight pack
    base = np.zeros((128, WC), np.float32)
    vp_W1 = weights["vp_W1"]
    for k in range(4):
        base[:, VPW1 + 128 * k : VPW1 + 128 * (k + 1)] = vp_W1[128 * k : 128 * (k + 1)]
    base[:, VPW2 : VPW2 + 128] = weights["vp_W2"]
    base[:, VPW3] = weights["vp_W3"][:, 0]
    base[0, SPW1 : SPW1 + 128] = weights["sp_W1"][0]
    base[:, SPW2 : SPW2 + 128] = weights["sp_W2"]
    base[:, SPW3 : SPW3 + 128] = weights["sp_W3"]
    base[:, SPB1] = weights["sp_b1"]
    base[:, SPB2] = weights["sp_b2"]
    base[:, SPB3] = weights["sp_b3"]
    base[:, VPB1] = weights["vp_b1"]
    base[:, VPB2] = weights["vp_b2"]
    base[0, VPB3] = weights["vp_b3"][0]

    # routing: expert e -> cores 2e, 2e+1
    command = np.asarray(command).astype(np.int64).ravel()
    order = np.argsort(command, kind="stable")
    counts = np.bincount(command, minlength=NEXPERT)
    starts = np.concatenate([[0], np.cumsum(counts)])

    core_idx = np.zeros((NCORES, RPC), np.int64)
    core_n = np.zeros(NCORES, np.int64)
    overflow = []
    for e in range(NEXPERT):
        rows = order[starts[e] : starts[e + 1]]
        if len(rows) > 2 * RPC:
            overflow.append(rows[2 * RPC :])
            rows = rows[: 2 * RPC]
        h = min((len(rows) + 1) // 2, RPC)
        for ci, part in ((2 * e, rows[:h]), (2 * e + 1, rows[h:])):
            core_idx[ci, : len(part)] = part
            core_n[ci] = len(part)

    in_maps = []
    for ci in range(NCORES):
        idx = core_idx[ci]
        e = ci // 2
        wp = base.copy()
        for k in range(5):
            wp[:, W1F + 256 * k : W1F + 256 * (k + 1)] = w1f[e][128 * k : 128 * (k + 1)]
        W2 = weights["ctrl_W2"][e]
        for k in range(2):
            wp[:, W2F + 256 * k : W2F + 256 * (k + 1)] = W2[128 * k : 128 * (k + 1)]
        W3 = weights["ctrl_W3"][e]
        for k in range(2):
            wp[:, W3F + 3 * k : W3F + 3 * (k + 1)] = W3[128 * k : 128 * (k + 1)]
        wp[:, B1F0] = b1f[e][:128]
        wp[:, B1F1] = b1f[e][128:]
        wp[:, B2F0] = weights["ctrl_b2"][e][:128]
        wp[:, B2F1] = weights["ctrl_b2"][e][128:]
        wp[0:3, B3F] = weights["ctrl_b3"][e]

        xT = np.ascontiguousarray(p_i[idx].T.astype(np.float32, copy=False))
        spd = np.ascontiguousarray(speed[idx].astype(np.float32, copy=False))[None, :]
        in_maps.append({"xT": xT, "spd": spd, "wp": wp})

    return in_maps, core_idx, core_n, overflow


def _mlp3_np(x, W1, b1, W2, b2, W3, b3):
    h = np.maximum(x @ W1 + b1, 0.0)
    h = np.maximum(h @ W2 + b2, 0.0)
    return h @ W3 + b3


def _host_fallback(rows, p_i, speed, command, w, v_p, action):
    """Exact-math fallback for rows that overflow per-expert capacity."""
    x = p_i[rows].astype(np.float32)
    s = speed[rows].astype(np.float32)
    v = _mlp3_np(s[:, None], w["sp_W1"], w["sp_b1"], w["sp_W2"], w["sp_b2"], w["sp_W3"], w["sp_b3"])
    joined = np.concatenate([x, v], axis=1) @ w["join_W"] + w["join_b"]
    v_p[rows, 0] = _mlp3_np(x, w["vp_W1"], w["vp_b1"], w["vp_W2"], w["vp_b2"], w["vp_W3"], w["vp_b3"])[:, 0]
    for i, r in enumerate(rows):
        e = int(command[r])
        h = np.maximum(joined[i] @ w["ctrl_W1"][e] + w["ctrl_b1"][e], 0.0)
        h = np.maximum(h @ w["ctrl_W2"][e] + w["ctrl_b2"][e], 0.0)
        action[r] = h @ w["ctrl_W3"][e] + w["ctrl_b3"][e]


# ---------------------------------------------------------------------------
# Entry point
# ---------------------------------------------------------------------------
def kernel(p_i, speed, command, **weights):
    p_i = np.asarray(p_i)
    speed = np.asarray(speed)
    command = np.asarray(command)
    weights = {k: np.asarray(v) for k, v in weights.items()}

    in_maps, core_idx, core_n, overflow = _prepare(p_i, speed, command, weights)
    run = _get_runner()
    results = run(in_maps)

    v_p = np.zeros((B, 1), np.float32)
    action = np.zeros((B, 3), np.float32)
    for ci in range(NCORES):
        n = int(core_n[ci])
        if n == 0:
            continue
        idx = core_idx[ci, :n]
        v_p[idx, 0] = results[ci]["vp_out"][0, :n]
        action[idx] = results[ci]["act_out"][:, :n].T

    for rows in overflow:
        _host_fallback(rows, p_i, speed, command, weights, v_p, action)

    return v_p, action


# revision 14
# speedup vs baseline: 1311.4602x; 1311.4602x over previous
"""Trainium2 Bass kernel for the CILRS control module (moe_routing).

Strategy
--------
Host side (numpy, part of sharding):
  * ``joined = concat(p_i, v) @ join_W + join_b`` feeds ONLY the 4
    control heads (no nonlinearity in between), and ``v`` (the speed
    embedding) feeds ONLY ``joined``.  So both the join layer and the
    last speed-MLP layer fold into the head's first weight:
        W1f[e] = [ join_W[:512] @ ctrl_W1[e]            ]   (k-tiles 0-3)
                 [ sp_W3 @ join_W[512:] @ ctrl_W1[e]    ]   (k-tile 4)
        b1f[e] = (sp_b3 @ join_W[512:] + join_b) @ ctrl_W1[e] + ctrl_b1[e]
    The on-device speed MLP shrinks to 2 layers (relu(W1*s+b1),
    relu(W2*h+b2)) whose output h2 is the k-tile-4 operand.
  * Rows are routed by ``command``: expert e's rows go to cores 2e and
    2e+1 (capacity 4608 rows/core, padded).  Each core receives only
    its expert's weights, so the device program is expert-agnostic.
  * p_i rows are gathered + transposed on the host so the device gets
    feature-major [512, 4608] activations (matmul contraction dim must
    sit on SBUF partitions).

Device side (per core, 4608 rows in 9 chunks of 512):
  fp32r matmuls (full PE rate at N=512), bias+ReLU epilogues fused into
  single ACT/DVE instructions reading PSUM directly.  The tiny speed-MLP
  weights load first so the PE has work while the big DMAs stream.
"""

import numpy as np

B = 32768
NCORES = 8
NEXPERT = 4
RPC = 4608            # rows per core (padded capacity)
CHUNK = 512
NCHUNK = RPC // CHUNK  # 9

# --- small pack (speed MLP): loads first ------------------------------------
SPW1 = 0                    # sp_W1 [1,128] (row 0 only)
SPW2 = SPW1 + 128           # sp_W2 [128,128]
SPB1 = SPW2 + 128
SPB2 = SPB1 + 1
SPC = SPB2 + 1              # = 258

# --- main pack --------------------------------------------------------------
VPW1 = 0                    # vp_W1 [512,128] -> 4 k-tiles of [128,128]
VPW2 = VPW1 + 4 * 128       # vp_W2 [128,128]
VPW3 = VPW2 + 128           # vp_W3 [128,1]
W1F = VPW3 + 1              # fused head W1f [640,256] -> 5 k-tiles of [128,256]
W2F = W1F + 5 * 256         # head W2 [256,256] -> 2 k-tiles of [128,256]
W3F = W2F + 2 * 256         # head W3 [256,3]  -> 2 k-tiles of [128,3]
VPB1 = W3F + 2 * 3
VPB2 = VPB1 + 1
VPB3 = VPB2 + 1             # row 0 only
B1F0 = VPB3 + 1             # fused head b1f[0:128]
B1F1 = B1F0 + 1             # fused head b1f[128:256]
B2F0 = B1F1 + 1
B2F1 = B2F0 + 1
B3F = B2F1 + 1              # rows 0..2
RC = B3F + 1

_CACHE = {}


# ---------------------------------------------------------------------------
# Device program
# ---------------------------------------------------------------------------
def _build_bass():
    import concourse.bacc as bacc
    import concourse.mybir as mybir
    import concourse.tile as tile

    f32r = mybir.dt.float32r
    f32 = mybir.dt.float32
    RELU = mybir.ActivationFunctionType.Relu
    ADD = mybir.AluOpType.add
    MAX = mybir.AluOpType.max

    nc = bacc.Bacc(
        "TRN2",
        target_bir_lowering=False,
        debug=False,
        enable_asserts=False,
        num_devices=NCORES,
    )
    xT_d = nc.dram_tensor("xT", [512, RPC], f32r, kind="ExternalInput")
    spd_d = nc.dram_tensor("spd", [1, RPC], f32r, kind="ExternalInput")
    wps_d = nc.dram_tensor("wps", [128, SPC], f32r, kind="ExternalInput")
    wpr_d = nc.dram_tensor("wpr", [128, RC], f32r, kind="ExternalInput")
    vp_d = nc.dram_tensor("vp_out", [1, RPC], f32r, kind="ExternalOutput")
    act_d = nc.dram_tensor("act_out", [3, RPC], f32r, kind="ExternalOutput")

    with tile.TileContext(nc) as tc:
        with (
            tc.tile_pool(name="const", bufs=1) as constp,
            tc.tile_pool(name="io", bufs=4) as iop,
            tc.tile_pool(name="spv", bufs=1) as spvp,
            tc.tile_pool(name="acts", bufs=3) as actp,
            tc.tile_pool(name="outs", bufs=1) as outp,
            tc.tile_pool(name="ps", bufs=8, space="PSUM") as psp,
        ):
            # DMA order on the sync ring: tiny sp pack + speed first so the
            # PE can start immediately; then the big packs stream under the
            # speed-MLP phase.
            wps = constp.tile([128, SPC], f32r)
            nc.sync.dma_start(wps[:], wps_d.ap())
            spd = constp.tile([1, RPC], f32r)
            nc.sync.dma_start(spd[:], spd_d.ap())
            wpr = constp.tile([128, RC], f32r)
            nc.sync.dma_start(wpr[:], wpr_d.ap())

            vp_all = outp.tile([1, RPC], f32r)
            act_all = outp.tile([3, RPC], f32r)

            xTr = xT_d.ap().rearrange("(k p) r -> p k r", p=128)

            def bcol(t, col, p0=0, pn=128):
                return t[p0:pn, col : col + 1].bitcast(f32)

            def relu_act(out, psum, t, bias_col):
                nc.scalar.activation(out, psum, RELU, bias=bcol(t, bias_col))

            def relu_dve(out, psum, t, bias_col):
                nc.vector.tensor_scalar(out, psum, bcol(t, bias_col), 0.0, ADD, MAX)

            # ---- software pipeline over row chunks ------------------------
            # Tick t emits: sp1(t+3), sp2(t+2), C1(t) [vp l1, head l1, vp l2],
            # vp3(t-1), W2(t-2), W3(t-3).  The lags keep every PE matmul's
            # ACT/DVE-epilogue dependency several ticks old, so the static
            # PE order never waits on a just-issued epilogue.
            h1ss = {}
            h2s = {}
            hv2s = {}
            h1ps = {}
            h2ps = {}
            xt_of = {}
            DMA_GROUPS = {0: [0], 1: [1], 2: [2, 3], 4: [4, 5], 6: [6, 7], 8: [8]}

            def emit_sp1(c):
                ps1 = psp.tile([128, CHUNK], f32, tag="ps")
                nc.tensor.matmul(
                    ps1[:], wps[0:1, SPW1 : SPW1 + 128],
                    spd[:, c * CHUNK : (c + 1) * CHUNK],
                    start=True, stop=True,
                )
                h1s = actp.tile([128, CHUNK], f32r, name="h1s", bufs=9)
                relu_act(h1s[:], ps1[:], wps, SPB1)
                h1ss[c] = h1s

            def emit_sp2(c):
                ps2 = psp.tile([128, CHUNK], f32, tag="ps")
                nc.tensor.matmul(
                    ps2[:], wps[:, SPW2 : SPW2 + 128], h1ss.pop(c)[:],
                    start=True, stop=True,
                )
                ht = spvp.tile([128, CHUNK], f32r, name=f"h2s{c}")
                relu_dve(ht[:], ps2[:], wps, SPB2)
                h2s[c] = ht

            def emit_c1(c):
                xt, half = xt_of[c]

                def xk(k):
                    return xt[:, k, half * CHUNK : (half + 1) * CHUNK]

                psv1 = psp.tile([128, CHUNK], f32, tag="ps")
                for k in range(4):
                    nc.tensor.matmul(
                        psv1[:],
                        wpr[:, VPW1 + 128 * k : VPW1 + 128 * (k + 1)],
                        xk(k),
                        start=(k == 0),
                        stop=(k == 3),
                    )
                hv1 = actp.tile([128, CHUNK], f32r, name="hv1", bufs=3)
                relu_act(hv1[:], psv1[:], wpr, VPB1)

                h1p = []
                for m in range(2):
                    psh1 = psp.tile([128, CHUNK], f32, tag="ps", name=f"psh1_{m}")
                    for k in range(4):
                        nc.tensor.matmul(
                            psh1[:],
                            wpr[:, W1F + 256 * k + 128 * m : W1F + 256 * k + 128 * (m + 1)],
                            xk(k),
                            start=(k == 0),
                            stop=False,
                        )
                    nc.tensor.matmul(
                        psh1[:],
                        wpr[:, W1F + 256 * 4 + 128 * m : W1F + 256 * 4 + 128 * (m + 1)],
                        h2s.pop(c)[:] if m == 1 else h2s[c][:],
                        start=False,
                        stop=True,
                    )
                    ht = actp.tile([128, CHUNK], f32r, name=f"h1p{m}", bufs=3)
                    if m == 0:
                        relu_act(ht[:], psh1[:], wpr, B1F0)
                    else:
                        relu_dve(ht[:], psh1[:], wpr, B1F1)
                    h1p.append(ht)
                h1ps[c] = h1p

                psv2 = psp.tile([128, CHUNK], f32, tag="ps")
                nc.tensor.matmul(
                    psv2[:], wpr[:, VPW2 : VPW2 + 128], hv1[:], start=True, stop=True
                )
                hv2 = actp.tile([128, CHUNK], f32r, name="hv2", bufs=3)
                relu_dve(hv2[:], psv2[:], wpr, VPB2)
                hv2s[c] = hv2

            def emit_vp3(c):
                sl = slice(c * CHUNK, (c + 1) * CHUNK)
                psv3 = psp.tile([1, CHUNK], f32, tag="ps")
                nc.tensor.matmul(
                    psv3[:], wpr[:, VPW3 : VPW3 + 1], hv2s.pop(c)[:],
                    start=True, stop=True,
                )
                nc.vector.tensor_scalar_add(
                    vp_all[0:1, sl], psv3[:], bcol(wpr, VPB3, 0, 1)
                )

            def emit_w2(c):
                h1p = h1ps.pop(c)
                h2p = []
                for m in range(2):
                    psh2 = psp.tile([128, CHUNK], f32, tag="ps", name=f"psh2_{m}")
                    for k in range(2):
                        nc.tensor.matmul(
                            psh2[:],
                            wpr[:, W2F + 256 * k + 128 * m : W2F + 256 * k + 128 * (m + 1)],
                            h1p[k][:],
                            start=(k == 0),
                            stop=(k == 1),
                        )
                    ht2 = actp.tile([128, CHUNK], f32r, name=f"h2p{m}", bufs=3)
                    relu_act(ht2[:], psh2[:], wpr, B2F0 + m)
                    h2p.append(ht2)
                h2ps[c] = h2p

            def emit_w3(c):
                sl = slice(c * CHUNK, (c + 1) * CHUNK)
                h2p = h2ps.pop(c)
                psh3 = psp.tile([3, CHUNK], f32, tag="ps")
                for k in range(2):
                    nc.tensor.matmul(
                        psh3[:],
                        wpr[:, W3F + 3 * k : W3F + 3 * (k + 1)],
                        h2p[k][:],
                        start=(k == 0),
                        stop=(k == 1),
                    )
                nc.vector.tensor_scalar_add(
                    act_all[0:3, sl], psh3[:], bcol(wpr, B3F, 0, 3)
                )

            for t in range(NCHUNK + 3):
                grp = DMA_GROUPS.get(t)
                if grp is not None:
                    n = len(grp)
                    name = "xt1" if n == 1 else "xt2"
                    xt = iop.tile([128, 4, n * CHUNK], f32r, name=name,
                                  bufs=2)
                    nc.sync.dma_start(
                        xt[:], xTr[:, :, grp[0] * CHUNK : (grp[-1] + 1) * CHUNK]
                    )
                    for i, cc in enumerate(grp):
                        xt_of[cc] = (xt, i)
                if t == 0:
                    for c in range(NCHUNK):
                        emit_sp1(c)
                    for c in range(min(3, NCHUNK)):
                        emit_sp2(c)
                if t + 3 < NCHUNK:
                    emit_sp2(t + 3)
                if t < NCHUNK:
                    emit_c1(t)
                if 0 <= t - 1 < NCHUNK:
                    emit_vp3(t - 1)
                if 0 <= t - 2 < NCHUNK:
                    emit_w2(t - 2)
                if 0 <= t - 3 < NCHUNK:
                    emit_w3(t - 3)
                if t - 3 == 2:
                    nc.gpsimd.dma_start(
                        act_d.ap()[:, : 3 * CHUNK], act_all[0:3, : 3 * CHUNK]
                    )
                if t - 3 == 5:
                    nc.gpsimd.dma_start(
                        act_d.ap()[:, 3 * CHUNK : 6 * CHUNK],
                        act_all[0:3, 3 * CHUNK : 6 * CHUNK],
                    )
                if t - 3 == 7:
                    nc.gpsimd.dma_start(
                        act_d.ap()[:, 6 * CHUNK : 8 * CHUNK],
                        act_all[0:3, 6 * CHUNK : 8 * CHUNK],
                    )
                if t - 1 == 4:
                    nc.gpsimd.dma_start(
                        vp_d.ap()[:, : 5 * CHUNK], vp_all[0:1, : 5 * CHUNK]
                    )
                if t - 1 == 7:
                    nc.gpsimd.dma_start(
                        vp_d.ap()[:, 5 * CHUNK : 8 * CHUNK],
                        vp_all[0:1, 5 * CHUNK : 8 * CHUNK],
                    )

            nc.sync.dma_start(vp_d.ap()[:, 8 * CHUNK :], vp_all[0:1, 8 * CHUNK :])
            nc.scalar.dma_start(
                act_d.ap()[:, 8 * CHUNK :], act_all[0:3, 8 * CHUNK :]
            )

    nc.compile()
    return nc


def _get_nc():
    if "nc" not in _CACHE:
        _CACHE["nc"] = _build_bass()
    return _CACHE["nc"]


# ---------------------------------------------------------------------------
# PJRT runner (axon): jit once, execute cheaply on repeat calls.
# ---------------------------------------------------------------------------
def _get_runner():
    if "runner" in _CACHE:
        return _CACHE["runner"]

    import jax
    import concourse.mybir as mybir
    from concourse import bass2jax
    from jax.experimental.shard_map import shard_map
    from jax.sharding import Mesh, PartitionSpec

    nc = _get_nc()
    bass2jax.install_neuronx_cc_hook()

    partition_name = nc.partition_id_tensor.name if nc.partition_id_tensor else None
    in_names = []
    out_names = []
    out_avals = []
    for alloc in nc.m.functions[0].allocations:
        if not isinstance(alloc, mybir.MemoryLocationSet):
            continue
        name = alloc.memorylocations[0].name
        if alloc.kind == "ExternalInput":
            if name != partition_name:
                in_names.append(name)
        elif alloc.kind == "ExternalOutput":
            shape = tuple(alloc.tensor_shape)
            dtype = mybir.dt.np(alloc.dtype)
            out_names.append(name)
            out_avals.append(jax.core.ShapedArray(shape, dtype))
    n_params = len(in_names)
    n_outs = len(out_avals)
    all_in_names = in_names + out_names
    if partition_name is not None:
        all_in_names = all_in_names + [partition_name]

    def _body(*args):
        operands = list(args)
        if partition_name is not None:
            operands.append(bass2jax.partition_id_tensor())
        outs = bass2jax._bass_exec_p.bind(
            *operands,
            out_avals=tuple(out_avals),
            in_names=tuple(all_in_names),
            out_names=tuple(out_names),
            lowering_input_output_aliases=(),
            sim_require_finite=True,
            sim_require_nnan=True,
            nc=nc,
        )
        return tuple(outs)

    devices = jax.devices()[:NCORES]
    mesh = Mesh(np.asarray(devices), ("core",))
    donate = tuple(range(n_params, n_params + n_outs))
    sharded = jax.jit(
        shard_map(
            _body,
            mesh=mesh,
            in_specs=(PartitionSpec("core"),) * (n_params + n_outs),
            out_specs=(PartitionSpec("core"),) * n_outs,
            check_rep=False,
        ),
        donate_argnums=donate,
        keep_unused=True,
    )

    _CACHE["sharded"] = sharded
    _CACHE["in_names"] = in_names
    _CACHE["out_avals"] = out_avals

    def run(in_maps):
        concat_in = [
            np.concatenate([in_maps[c][name] for c in range(NCORES)], axis=0)
            for name in in_names
        ]
        concat_zeros = [
            np.zeros((NCORES * a.shape[0], *a.shape[1:]), a.dtype) for a in out_avals
        ]
        out_arrs = sharded(*concat_in, *concat_zeros)
        return [
            {
                name: np.asarray(out_arrs[i]).reshape(NCORES, *out_avals[i].shape)[c]
                for i, name in enumerate(out_names)
            }
            for c in range(NCORES)
        ]

    _CACHE["runner"] = run
    return run


# ---------------------------------------------------------------------------
# Host-side prep: weight folding, routing, packing
# ---------------------------------------------------------------------------
def _prepare(p_i, speed, command, weights):
    f64 = np.float64
    join_W = weights["join_W"].astype(f64)
    join_b = weights["join_b"].astype(f64)
    sp_W3 = weights["sp_W3"].astype(f64)
    sp_b3 = weights["sp_b3"].astype(f64)
    Jp = join_W[:512]
    Jv = join_W[512:]
    w1f = []
    b1f = []
    for e in range(NEXPERT):
        c1 = weights["ctrl_W1"][e].astype(f64)
        top = Jp @ c1                      # [512, 256]
        bot = sp_W3 @ (Jv @ c1)            # [128, 256]
        w1f.append(np.concatenate([top, bot], axis=0).astype(np.float32))
        b1f.append(
            ((sp_b3 @ Jv + join_b) @ c1 + weights["ctrl_b1"][e].astype(f64)).astype(
                np.float32
            )
        )

    wps = np.zeros((128, SPC), np.float32)
    wps[0, SPW1 : SPW1 + 128] = weights["sp_W1"][0]
    wps[:, SPW2 : SPW2 + 128] = weights["sp_W2"]
    wps[:, SPB1] = weights["sp_b1"]
    wps[:, SPB2] = weights["sp_b2"]

    base = np.zeros((128, RC), np.float32)
    vp_W1 = weights["vp_W1"]
    for k in range(4):
        base[:, VPW1 + 128 * k : VPW1 + 128 * (k + 1)] = vp_W1[128 * k : 128 * (k + 1)]
    base[:, VPW2 : VPW2 + 128] = weights["vp_W2"]
    base[:, VPW3] = weights["vp_W3"][:, 0]
    base[:, VPB1] = weights["vp_b1"]
    base[:, VPB2] = weights["vp_b2"]
    base[0, VPB3] = weights["vp_b3"][0]

    # routing: expert e -> cores 2e, 2e+1
    command = np.asarray(command).astype(np.int64).ravel()
    order = np.argsort(command, kind="stable")
    counts = np.bincount(command, minlength=NEXPERT)
    starts = np.concatenate([[0], np.cumsum(counts)])

    core_idx = np.zeros((NCORES, RPC), np.int64)
    core_n = np.zeros(NCORES, np.int64)
    overflow = []
    for e in range(NEXPERT):
        rows = order[starts[e] : starts[e + 1]]
        if len(rows) > 2 * RPC:
            overflow.append(rows[2 * RPC :])
            rows = rows[: 2 * RPC]
        h = min((len(rows) + 1) // 2, RPC)
        for ci, part in ((2 * e, rows[:h]), (2 * e + 1, rows[h:])):
            core_idx[ci, : len(part)] = part
            core_n[ci] = len(part)

    in_maps = []
    for ci in range(NCORES):
        idx = core_idx[ci]
        e = ci // 2
        wpr = base.copy()
        for k in range(5):
            wpr[:, W1F + 256 * k : W1F + 256 * (k + 1)] = w1f[e][128 * k : 128 * (k + 1)]
        W2 = weights["ctrl_W2"][e]
        for k in range(2):
            wpr[:, W2F + 256 * k : W2F + 256 * (k + 1)] = W2[128 * k : 128 * (k + 1)]
        W3 = weights["ctrl_W3"][e]
        for k in range(2):
            wpr[:, W3F + 3 * k : W3F + 3 * (k + 1)] = W3[128 * k : 128 * (k + 1)]
        wpr[:, B1F0] = b1f[e][:128]
        wpr[:, B1F1] = b1f[e][128:]
        wpr[:, B2F0] = weights["ctrl_b2"][e][:128]
        wpr[:, B2F1] = weights["ctrl_b2"][e][128:]
        wpr[0:3, B3F] = weights["ctrl_b3"][e]

        xT = np.ascontiguousarray(p_i[idx].T.astype(np.float32, copy=False))
        spd = np.ascontiguousarray(speed[idx].astype(np.float32, copy=False))[None, :]
        in_maps.append({"xT": xT, "spd": spd, "wps": wps, "wpr": wpr})

    return in_maps, core_idx, core_n, overflow


def _mlp3_np(x, W1, b1, W2, b2, W3, b3):
    h = np.maximum(x @ W1 + b1, 0.0)
    h = np.maximum(h @ W2 + b2, 0.0)
    return h @ W3 + b3


def _host_fallback(rows, p_i, speed, command, w, v_p, action):
    """Exact-math fallback for rows that overflow per-expert capacity."""
    x = p_i[rows].astype(np.float32)
    s = speed[rows].astype(np.float32)
    v = _mlp3_np(s[:, None], w["sp_W1"], w["sp_b1"], w["sp_W2"], w["sp_b2"], w["sp_W3"], w["sp_b3"])
    joined = np.concatenate([x, v], axis=1) @ w["join_W"] + w["join_b"]
    v_p[rows, 0] = _mlp3_np(x, w["vp_W1"], w["vp_b1"], w["vp_W2"], w["vp_b2"], w["vp_W3"], w["vp_b3"])[:, 0]
    for i, r in enumerate(rows):
        e = int(command[r])
        h = np.maximum(joined[i] @ w["ctrl_W1"][e] + w["ctrl_b1"][e], 0.0)
        h = np.maximum(h @ w["ctrl_W2"][e] + w["ctrl_b2"][e], 0.0)
        action[r] = h @ w["ctrl_W3"][e] + w["ctrl_b3"][e]


# ---------------------------------------------------------------------------
# Entry point
# ---------------------------------------------------------------------------
def kernel(p_i, speed, command, **weights):
    p_i = np.asarray(p_i)
    speed = np.asarray(speed)
    command = np.asarray(command)
    weights = {k: np.asarray(v) for k, v in weights.items()}

    in_maps, core_idx, core_n, overflow = _prepare(p_i, speed, command, weights)
    run = _get_runner()
    results = run(in_maps)

    v_p = np.zeros((B, 1), np.float32)
    action = np.zeros((B, 3), np.float32)
    for ci in range(NCORES):
        n = int(core_n[ci])
        if n == 0:
            continue
        idx = core_idx[ci, :n]
        v_p[idx, 0] = results[ci]["vp_out"][0, :n]
        action[idx] = results[ci]["act_out"][:, :n].T

    for rows in overflow:
        _host_fallback(rows, p_i, speed, command, weights, v_p, action)

    return v_p, action


# revision 16
# speedup vs baseline: 31089.2465x; 23.7058x over previous
"""Trainium2 Bass kernel for the CILRS control module (moe_routing).

Strategy
--------
Host side (numpy, part of sharding):
  * ``joined = concat(p_i, v) @ join_W + join_b`` feeds ONLY the 4
    control heads (no nonlinearity in between), and ``v`` (the speed
    embedding) feeds ONLY ``joined``.  So both the join layer and the
    last speed-MLP layer fold into the head's first weight:
        W1f[e] = [ join_W[:512] @ ctrl_W1[e]            ]   (k-tiles 0-3)
                 [ sp_W3 @ join_W[512:] @ ctrl_W1[e]    ]   (k-tile 4)
        b1f[e] = (sp_b3 @ join_W[512:] + join_b) @ ctrl_W1[e] + ctrl_b1[e]
    The on-device speed MLP shrinks to 2 layers (relu(W1*s+b1),
    relu(W2*h+b2)) whose output h2 is the k-tile-4 operand.
  * Rows are routed by ``command``: expert e's rows go to cores 2e and
    2e+1 (capacity 4608 rows/core, padded).  Each core receives only
    its expert's weights, so the device program is expert-agnostic.
  * p_i rows are gathered + transposed on the host so the device gets
    feature-major [512, 4608] activations (matmul contraction dim must
    sit on SBUF partitions).

Device side (per core, 4608 rows in 9 chunks of 512):
  fp32r matmuls (full PE rate at N=512), bias+ReLU epilogues fused into
  single ACT/DVE instructions reading PSUM directly.  The tiny speed-MLP
  weights load first so the PE has work while the big DMAs stream.
"""

import numpy as np

B = 32768
NCORES = 8
NEXPERT = 4
RPC = 4608            # rows per core (padded capacity)
CHUNK = 512
NCHUNK = RPC // CHUNK  # 9

# --- small pack (speed MLP): loads first ------------------------------------
SPW1 = 0                    # sp_W1 [1,128] (row 0 only)
SPW2 = SPW1 + 128           # sp_W2 [128,128]
SPB1 = SPW2 + 128
SPB2 = SPB1 + 1
SPC = SPB2 + 1              # = 258

# --- main pack --------------------------------------------------------------
VPW1 = 0                    # vp_W1 [512,128] -> 4 k-tiles of [128,128]
VPW2 = VPW1 + 4 * 128       # vp_W2 [128,128]
VPW3 = VPW2 + 128           # vp_W3 [128,1]
W1F = VPW3 + 1              # fused head W1f [640,256] -> 5 k-tiles of [128,256]
W2F = W1F + 5 * 256         # head W2 [256,256] -> 2 k-tiles of [128,256]
W3F = W2F + 2 * 256         # head W3 [256,3]  -> 2 k-tiles of [128,3]
VPB1 = W3F + 2 * 3
VPB2 = VPB1 + 1
VPB3 = VPB2 + 1             # row 0 only
B1F0 = VPB3 + 1             # fused head b1f[0:128]
B1F1 = B1F0 + 1             # fused head b1f[128:256]
B2F0 = B1F1 + 1
B2F1 = B2F0 + 1
B3F = B2F1 + 1              # rows 0..2
RC = B3F + 1

_CACHE = {}


# ---------------------------------------------------------------------------
# Device program
# ---------------------------------------------------------------------------
def _build_bass(reps=1):
    import concourse.bacc as bacc
    import concourse.mybir as mybir
    import concourse.tile as tile

    f32r = mybir.dt.float32r
    f32 = mybir.dt.float32
    RELU = mybir.ActivationFunctionType.Relu
    ADD = mybir.AluOpType.add
    MAX = mybir.AluOpType.max

    nc = bacc.Bacc(
        "TRN2",
        target_bir_lowering=False,
        debug=False,
        enable_asserts=False,
        num_devices=NCORES,
    )
    xT_d = nc.dram_tensor("xT", [512, RPC], f32r, kind="ExternalInput")
    spd_d = nc.dram_tensor("spd", [1, RPC], f32r, kind="ExternalInput")
    wps_d = nc.dram_tensor("wps", [128, SPC], f32r, kind="ExternalInput")
    wpr_d = nc.dram_tensor("wpr", [128, RC], f32r, kind="ExternalInput")
    vp_d = nc.dram_tensor("vp_out", [1, RPC], f32r, kind="ExternalOutput")
    act_d = nc.dram_tensor("act_out", [3, RPC], f32r, kind="ExternalOutput")

    with tile.TileContext(nc) as tc:
        with (
            tc.tile_pool(name="const", bufs=1) as constp,
            tc.tile_pool(name="io", bufs=4) as iop,
            tc.tile_pool(name="spv", bufs=1) as spvp,
            tc.tile_pool(name="acts", bufs=3) as actp,
            tc.tile_pool(name="outs", bufs=1) as outp,
            tc.tile_pool(name="ps", bufs=8, space="PSUM") as psp,
        ):
          for _rep in range(reps):
            # DMA order on the sync ring: tiny sp pack + speed first so the
            # PE can start immediately; then the big packs stream under the
            # speed-MLP phase.
            wps = constp.tile([128, SPC], f32r)
            nc.sync.dma_start(wps[:], wps_d.ap())
            spd = constp.tile([1, RPC], f32r)
            nc.sync.dma_start(spd[:], spd_d.ap())
            wpr = constp.tile([128, RC], f32r)
            nc.sync.dma_start(wpr[:], wpr_d.ap())

            vp_all = outp.tile([1, RPC], f32r)
            act_all = outp.tile([3, RPC], f32r)

            xTr = xT_d.ap().rearrange("(k p) r -> p k r", p=128)

            def bcol(t, col, p0=0, pn=128):
                return t[p0:pn, col : col + 1].bitcast(f32)

            def relu_act(out, psum, t, bias_col):
                nc.scalar.activation(out, psum, RELU, bias=bcol(t, bias_col))

            def relu_dve(out, psum, t, bias_col):
                nc.vector.tensor_scalar(out, psum, bcol(t, bias_col), 0.0, ADD, MAX)

            # ---- software pipeline over row chunks ------------------------
            # Tick t emits: sp1(t+3), sp2(t+2), C1(t) [vp l1, head l1, vp l2],
            # vp3(t-1), W2(t-2), W3(t-3).  The lags keep every PE matmul's
            # ACT/DVE-epilogue dependency several ticks old, so the static
            # PE order never waits on a just-issued epilogue.
            h1ss = {}
            h2s = {}
            hv2s = {}
            h1ps = {}
            h2ps = {}
            xt_of = {}
            DMA_GROUPS = {0: [0], 1: [1], 2: [2, 3], 4: [4, 5], 6: [6, 7], 8: [8]}

            def emit_sp1(c):
                ps1 = psp.tile([128, CHUNK], f32, tag="ps")
                nc.tensor.matmul(
                    ps1[:], wps[0:1, SPW1 : SPW1 + 128],
                    spd[:, c * CHUNK : (c + 1) * CHUNK],
                    start=True, stop=True,
                )
                h1s = actp.tile([128, CHUNK], f32r, name="h1s", bufs=9)
                relu_act(h1s[:], ps1[:], wps, SPB1)
                h1ss[c] = h1s

            def emit_sp2(c):
                ps2 = psp.tile([128, CHUNK], f32, tag="ps")
                nc.tensor.matmul(
                    ps2[:], wps[:, SPW2 : SPW2 + 128], h1ss.pop(c)[:],
                    start=True, stop=True,
                )
                ht = spvp.tile([128, CHUNK], f32r, name=f"h2s{c}", tag="h2s", bufs=5)
                relu_dve(ht[:], ps2[:], wps, SPB2)
                h2s[c] = ht

            def emit_c1(c):
                xt, half = xt_of[c]

                def xk(k):
                    return xt[:, k, half * CHUNK : (half + 1) * CHUNK]

                psv1 = psp.tile([128, CHUNK], f32, tag="ps")
                for k in range(4):
                    nc.tensor.matmul(
                        psv1[:],
                        wpr[:, VPW1 + 128 * k : VPW1 + 128 * (k + 1)],
                        xk(k),
                        start=(k == 0),
                        stop=(k == 3),
                    )
                hv1 = actp.tile([128, CHUNK], f32r, name="hv1", bufs=2)
                relu_act(hv1[:], psv1[:], wpr, VPB1)

                h1p = []
                for m in range(2):
                    psh1 = psp.tile([128, CHUNK], f32, tag="ps", name=f"psh1_{m}")
                    for k in range(4):
                        nc.tensor.matmul(
                            psh1[:],
                            wpr[:, W1F + 256 * k + 128 * m : W1F + 256 * k + 128 * (m + 1)],
                            xk(k),
                            start=(k == 0),
                            stop=False,
                        )
                    nc.tensor.matmul(
                        psh1[:],
                        wpr[:, W1F + 256 * 4 + 128 * m : W1F + 256 * 4 + 128 * (m + 1)],
                        h2s.pop(c)[:] if m == 1 else h2s[c][:],
                        start=False,
                        stop=True,
                    )
                    ht = actp.tile([128, CHUNK], f32r, name=f"h1p{m}", bufs=3)
                    if m == 0:
                        relu_act(ht[:], psh1[:], wpr, B1F0)
                    else:
                        relu_dve(ht[:], psh1[:], wpr, B1F1)
                    h1p.append(ht)
                h1ps[c] = h1p

                psv2 = psp.tile([128, CHUNK], f32, tag="ps")
                nc.tensor.matmul(
                    psv2[:], wpr[:, VPW2 : VPW2 + 128], hv1[:], start=True, stop=True
                )
                hv2 = actp.tile([128, CHUNK], f32r, name="hv2", bufs=3)
                relu_dve(hv2[:], psv2[:], wpr, VPB2)
                hv2s[c] = hv2

            def emit_vp3(c):
                sl = slice(c * CHUNK, (c + 1) * CHUNK)
                psv3 = psp.tile([1, CHUNK], f32, tag="ps")
                nc.tensor.matmul(
                    psv3[:], wpr[:, VPW3 : VPW3 + 1], hv2s.pop(c)[:],
                    start=True, stop=True,
                )
                nc.vector.tensor_scalar_add(
                    vp_all[0:1, sl], psv3[:], bcol(wpr, VPB3, 0, 1)
                )

            def emit_w2(c):
                h1p = h1ps.pop(c)
                h2p = []
                for m in range(2):
                    psh2 = psp.tile([128, CHUNK], f32, tag="ps", name=f"psh2_{m}")
                    for k in range(2):
                        nc.tensor.matmul(
                            psh2[:],
                            wpr[:, W2F + 256 * k + 128 * m : W2F + 256 * k + 128 * (m + 1)],
                            h1p[k][:],
                            start=(k == 0),
                            stop=(k == 1),
                        )
                    ht2 = actp.tile([128, CHUNK], f32r, name=f"h2p{m}", bufs=2)
                    relu_act(ht2[:], psh2[:], wpr, B2F0 + m)
                    h2p.append(ht2)
                h2ps[c] = h2p

            def emit_w3(c):
                sl = slice(c * CHUNK, (c + 1) * CHUNK)
                h2p = h2ps.pop(c)
                psh3 = psp.tile([3, CHUNK], f32, tag="ps")
                for k in range(2):
                    nc.tensor.matmul(
                        psh3[:],
                        wpr[:, W3F + 3 * k : W3F + 3 * (k + 1)],
                        h2p[k][:],
                        start=(k == 0),
                        stop=(k == 1),
                    )
                nc.vector.tensor_scalar_add(
                    act_all[0:3, sl], psh3[:], bcol(wpr, B3F, 0, 3)
                )

            GRP_RING = {0: "s", 1: "p", 2: "s", 4: "p", 6: "s", 8: "p"}
            for t in range(NCHUNK + 3):
                grp = DMA_GROUPS.get(t)
                if grp is not None:
                    n = len(grp)
                    ring = GRP_RING[t]
                    xt = iop.tile([128, 4, 2 * CHUNK], f32r, name=f"xt_{ring}",
                                  tag=f"xt{ring}", bufs=2)
                    dma_eng = nc.sync if ring == "s" else nc.gpsimd
                    dma_eng.dma_start(
                        xt[:, :, : n * CHUNK],
                        xTr[:, :, grp[0] * CHUNK : (grp[-1] + 1) * CHUNK],
                    )
                    for i, cc in enumerate(grp):
                        xt_of[cc] = (xt, i)
                if t == 0:
                    for c in range(NCHUNK):
                        emit_sp1(c)
                    for c in range(min(3, NCHUNK)):
                        emit_sp2(c)
                if t + 3 < NCHUNK:
                    emit_sp2(t + 3)
                if t < NCHUNK:
                    emit_c1(t)
                if 0 <= t - 1 < NCHUNK:
                    emit_vp3(t - 1)
                if 0 <= t - 2 < NCHUNK:
                    emit_w2(t - 2)
                if 0 <= t - 3 < NCHUNK:
                    emit_w3(t - 3)
                if t - 3 == 2:
                    nc.gpsimd.dma_start(
                        act_d.ap()[:, : 3 * CHUNK], act_all[0:3, : 3 * CHUNK]
                    )
                if t - 3 == 5:
                    nc.gpsimd.dma_start(
                        act_d.ap()[:, 3 * CHUNK : 6 * CHUNK],
                        act_all[0:3, 3 * CHUNK : 6 * CHUNK],
                    )
                if t - 3 == 7:
                    nc.gpsimd.dma_start(
                        act_d.ap()[:, 6 * CHUNK : 8 * CHUNK],
                        act_all[0:3, 6 * CHUNK : 8 * CHUNK],
                    )
                if t - 1 == 4:
                    nc.gpsimd.dma_start(
                        vp_d.ap()[:, : 5 * CHUNK], vp_all[0:1, : 5 * CHUNK]
                    )
                if t - 1 == 7:
                    nc.gpsimd.dma_start(
                        vp_d.ap()[:, 5 * CHUNK : 8 * CHUNK],
                        vp_all[0:1, 5 * CHUNK : 8 * CHUNK],
                    )

            nc.sync.dma_start(vp_d.ap()[:, 8 * CHUNK :], vp_all[0:1, 8 * CHUNK :])
            nc.scalar.dma_start(
                act_d.ap()[:, 8 * CHUNK :], act_all[0:3, 8 * CHUNK :]
            )

    nc.compile()
    return nc


def _get_nc(reps=1):
    key = f"nc{reps}"
    if key not in _CACHE:
        _CACHE[key] = _build_bass(reps)
    return _CACHE[key]


# ---------------------------------------------------------------------------
# PJRT runner (axon): jit once, execute cheaply on repeat calls.
# ---------------------------------------------------------------------------
def _get_runner():
    if "runner" in _CACHE:
        return _CACHE["runner"]

    import jax
    import concourse.mybir as mybir
    from concourse import bass2jax
    from jax.experimental.shard_map import shard_map
    from jax.sharding import Mesh, PartitionSpec

    nc = _get_nc()
    bass2jax.install_neuronx_cc_hook()

    partition_name = nc.partition_id_tensor.name if nc.partition_id_tensor else None
    in_names = []
    out_names = []
    out_avals = []
    for alloc in nc.m.functions[0].allocations:
        if not isinstance(alloc, mybir.MemoryLocationSet):
            continue
        name = alloc.memorylocations[0].name
        if alloc.kind == "ExternalInput":
            if name != partition_name:
                in_names.append(name)
        elif alloc.kind == "ExternalOutput":
            shape = tuple(alloc.tensor_shape)
            dtype = mybir.dt.np(alloc.dtype)
            out_names.append(name)
            out_avals.append(jax.core.ShapedArray(shape, dtype))
    n_params = len(in_names)
    n_outs = len(out_avals)
    all_in_names = in_names + out_names
    if partition_name is not None:
        all_in_names = all_in_names + [partition_name]

    def _body(*args):
        operands = list(args)
        if partition_name is not None:
            operands.append(bass2jax.partition_id_tensor())
        outs = bass2jax._bass_exec_p.bind(
            *operands,
            out_avals=tuple(out_avals),
            in_names=tuple(all_in_names),
            out_names=tuple(out_names),
            lowering_input_output_aliases=(),
            sim_require_finite=True,
            sim_require_nnan=True,
            nc=nc,
        )
        return tuple(outs)

    devices = jax.devices()[:NCORES]
    mesh = Mesh(np.asarray(devices), ("core",))
    donate = tuple(range(n_params, n_params + n_outs))
    sharded = jax.jit(
        shard_map(
            _body,
            mesh=mesh,
            in_specs=(PartitionSpec("core"),) * (n_params + n_outs),
            out_specs=(PartitionSpec("core"),) * n_outs,
            check_rep=False,
        ),
        donate_argnums=donate,
        keep_unused=True,
    )

    _CACHE["sharded"] = sharded
    _CACHE["in_names"] = in_names
    _CACHE["out_avals"] = out_avals

    def run(in_maps):
        concat_in = [
            np.concatenate([in_maps[c][name] for c in range(NCORES)], axis=0)
            for name in in_names
        ]
        concat_zeros = [
            np.zeros((NCORES * a.shape[0], *a.shape[1:]), a.dtype) for a in out_avals
        ]
        out_arrs = sharded(*concat_in, *concat_zeros)
        return [
            {
                name: np.asarray(out_arrs[i]).reshape(NCORES, *out_avals[i].shape)[c]
                for i, name in enumerate(out_names)
            }
            for c in range(NCORES)
        ]

    _CACHE["runner"] = run
    return run


# ---------------------------------------------------------------------------
# Host-side prep: weight folding, routing, packing
# ---------------------------------------------------------------------------
def _prepare(p_i, speed, command, weights):
    f64 = np.float64
    join_W = weights["join_W"].astype(f64)
    join_b = weights["join_b"].astype(f64)
    sp_W3 = weights["sp_W3"].astype(f64)
    sp_b3 = weights["sp_b3"].astype(f64)
    Jp = join_W[:512]
    Jv = join_W[512:]
    w1f = []
    b1f = []
    for e in range(NEXPERT):
        c1 = weights["ctrl_W1"][e].astype(f64)
        top = Jp @ c1                      # [512, 256]
        bot = sp_W3 @ (Jv @ c1)            # [128, 256]
        w1f.append(np.concatenate([top, bot], axis=0).astype(np.float32))
        b1f.append(
            ((sp_b3 @ Jv + join_b) @ c1 + weights["ctrl_b1"][e].astype(f64)).astype(
                np.float32
            )
        )

    wps = np.zeros((128, SPC), np.float32)
    wps[0, SPW1 : SPW1 + 128] = weights["sp_W1"][0]
    wps[:, SPW2 : SPW2 + 128] = weights["sp_W2"]
    wps[:, SPB1] = weights["sp_b1"]
    wps[:, SPB2] = weights["sp_b2"]

    base = np.zeros((128, RC), np.float32)
    vp_W1 = weights["vp_W1"]
    for k in range(4):
        base[:, VPW1 + 128 * k : VPW1 + 128 * (k + 1)] = vp_W1[128 * k : 128 * (k + 1)]
    base[:, VPW2 : VPW2 + 128] = weights["vp_W2"]
    base[:, VPW3] = weights["vp_W3"][:, 0]
    base[:, VPB1] = weights["vp_b1"]
    base[:, VPB2] = weights["vp_b2"]
    base[0, VPB3] = weights["vp_b3"][0]

    # routing: expert e -> cores 2e, 2e+1
    command = np.asarray(command).astype(np.int64).ravel()
    order = np.argsort(command, kind="stable")
    counts = np.bincount(command, minlength=NEXPERT)
    starts = np.concatenate([[0], np.cumsum(counts)])

    core_idx = np.zeros((NCORES, RPC), np.int64)
    core_n = np.zeros(NCORES, np.int64)
    overflow = []
    for e in range(NEXPERT):
        rows = order[starts[e] : starts[e + 1]]
        if len(rows) > 2 * RPC:
            overflow.append(rows[2 * RPC :])
            rows = rows[: 2 * RPC]
        h = min((len(rows) + 1) // 2, RPC)
        for ci, part in ((2 * e, rows[:h]), (2 * e + 1, rows[h:])):
            core_idx[ci, : len(part)] = part
            core_n[ci] = len(part)

    in_maps = []
    for ci in range(NCORES):
        idx = core_idx[ci]
        e = ci // 2
        wpr = base.copy()
        for k in range(5):
            wpr[:, W1F + 256 * k : W1F + 256 * (k + 1)] = w1f[e][128 * k : 128 * (k + 1)]
        W2 = weights["ctrl_W2"][e]
        for k in range(2):
            wpr[:, W2F + 256 * k : W2F + 256 * (k + 1)] = W2[128 * k : 128 * (k + 1)]
        W3 = weights["ctrl_W3"][e]
        for k in range(2):
            wpr[:, W3F + 3 * k : W3F + 3 * (k + 1)] = W3[128 * k : 128 * (k + 1)]
        wpr[:, B1F0] = b1f[e][:128]
        wpr[:, B1F1] = b1f[e][128:]
        wpr[:, B2F0] = weights["ctrl_b2"][e][:128]
        wpr[:, B2F1] = weights["ctrl_b2"][e][128:]
        wpr[0:3, B3F] = weights["ctrl_b3"][e]

        xT = np.ascontiguousarray(p_i[idx].T.astype(np.float32, copy=False))
        spd = np.ascontiguousarray(speed[idx].astype(np.float32, copy=False))[None, :]
        in_maps.append({"xT": xT, "spd": spd, "wps": wps, "wpr": wpr})

    return in_maps, core_idx, core_n, overflow


def _mlp3_np(x, W1, b1, W2, b2, W3, b3):
    h = np.maximum(x @ W1 + b1, 0.0)
    h = np.maximum(h @ W2 + b2, 0.0)
    return h @ W3 + b3


def _host_fallback(rows, p_i, speed, command, w, v_p, action):
    """Exact-math fallback for rows that overflow per-expert capacity."""
    x = p_i[rows].astype(np.float32)
    s = speed[rows].astype(np.float32)
    v = _mlp3_np(s[:, None], w["sp_W1"], w["sp_b1"], w["sp_W2"], w["sp_b2"], w["sp_W3"], w["sp_b3"])
    joined = np.concatenate([x, v], axis=1) @ w["join_W"] + w["join_b"]
    v_p[rows, 0] = _mlp3_np(x, w["vp_W1"], w["vp_b1"], w["vp_W2"], w["vp_b2"], w["vp_W3"], w["vp_b3"])[:, 0]
    for i, r in enumerate(rows):
        e = int(command[r])
        h = np.maximum(joined[i] @ w["ctrl_W1"][e] + w["ctrl_b1"][e], 0.0)
        h = np.maximum(h @ w["ctrl_W2"][e] + w["ctrl_b2"][e], 0.0)
        action[r] = h @ w["ctrl_W3"][e] + w["ctrl_b3"][e]


# ---------------------------------------------------------------------------
# Entry point
# ---------------------------------------------------------------------------
def kernel(p_i, speed, command, **weights):
    p_i = np.asarray(p_i)
    speed = np.asarray(speed)
    command = np.asarray(command)
    weights = {k: np.asarray(v) for k, v in weights.items()}

    in_maps, core_idx, core_n, overflow = _prepare(p_i, speed, command, weights)
    run = _get_runner()
    results = run(in_maps)

    v_p = np.zeros((B, 1), np.float32)
    action = np.zeros((B, 3), np.float32)
    for ci in range(NCORES):
        n = int(core_n[ci])
        if n == 0:
            continue
        idx = core_idx[ci, :n]
        v_p[idx, 0] = results[ci]["vp_out"][0, :n]
        action[idx] = results[ci]["act_out"][:, :n].T

    for rows in overflow:
        _host_fallback(rows, p_i, speed, command, weights, v_p, action)

    return v_p, action
